# revision 40
# baseline (speedup 1.0000x reference)
"""
Distributed brute-force kNN memory-critic kernel for Trainium2 (8 NeuronCores).

Math:
  reference computes, per query q (B=256), squared L2 distances to N=500000
  memory rows m (D=34), takes the 10 nearest, then softmax over the raw
  distances weighting mem_Q values.

  d2(q,m) = |q|^2 - 2 q.m + |m|^2.
  Both top-k selection and softmax(dists) are invariant to the per-query
  constant |q|^2, so the device works with the *negated shifted score*
      s'(q,m) = 2 q.m - |m|^2        (larger s' == closer neighbor)
  computed as one matmul with augmented operands qaug=[2q,1], maug=[m,-|m|^2].

Device (per core, 62500-row shard padded to 63488 cols):
  - fp16 operands, PE 64x128 row-tiling: query block 0 on tile T0
    (SBUF partitions 0-35), block 1 on T8 (64-99), matmuls interleaved so the
    two array tiles overlap.  Scores accumulate in fp32 PSUM.
  - drain/reduce: each [128, 1024] PSUM tile is reduced to per-32-column
    group maxima, either directly on the vector engine (segmented
    tensor_reduce) or via scalar-engine fp16 copy + vector fp16 max tree
    (2 elem/cycle), scheduled so ACT and DVE run concurrently.
  - per query, top-16 groups of each half of the group-max array are
    extracted with max8 / max_index / match_replace and DMA'd out.

Cover guarantee (exact): if x is among a core's true local top-10 scores in
some half, x's group max >= x >= (10th best of half) >= (10th largest group
max of the half), so x's group is inside that half's top-10 groups; we
return 16 per half.  The fp16 score error (~+-0.03) is far below the
measured 10th->17th group-max margin (>=0.67), verified on the fixed inputs.

Host epilogue: exactly rescore the candidate groups' rows in float64
(256 queries x 8 cores x 32 groups x 32 rows), exact global top-10,
softmax-weighted sum of mem_Q.  All 128M distance computations + the
10^8-element reduction happen on device; host does ~70M MACs of rescoring.
"""

import numpy as np

NCORES = 8
B = 256
D = 34
DAUG = 36
N = 500000
NS = N // NCORES          # 62500 rows per core
CHUNK = 1024              # supertile columns (2 PSUM banks)
NSUPER = 62
NSP = NSUPER * CHUNK      # 63488 padded shard columns
G = 32                    # group width for the segmented max
NGROUPS = NSP // G        # 1984 groups per 128-query block
GPS = CHUNK // G          # groups per supertile (32)
RUN = 4                   # supertiles batched per fp16 tree pass
TOPG = 16                 # groups returned per query per half
NEG_PAD = -1.0e30         # score assigned to padding columns
NEG_REP = -3.0e38         # match_replace fill

_CACHE = {}


def _schedule():
    """Drain routing + r-column emission order; must match _build_bass.

    Returns (events, group_map, splits):
      events: list of ("tr", b, st) | ("tree", b, [sts])
      group_map[b]: list of NGROUPS global group ids in r-column order
      splits[b]: r column where the extraction is split in two halves
    """
    events = []
    for st in range(NSUPER):
        events.append(("tr", 0, st) if st % 6 != 0 else ("stage", 0, st))
        events.append(("stage", 1, st))

    group_map = [[], []]
    splits = [None, None]
    emit = []
    pend = [[], []]
    for ev in events:
        kind, b, st = ev
        if kind == "tr":
            emit.append(("tr", b, [st]))
        else:
            pend[b].append(st)
            if len(pend[b]) == RUN:
                emit.append(("tree", b, list(pend[b])))
                pend[b] = []
    for b in range(2):
        if pend[b]:
            emit.append(("tree", b, list(pend[b])))
    for kind, b, sts in emit:
        for st in sts:
            group_map[b].extend(range(st * GPS, (st + 1) * GPS))
    # split points: first emission boundary at/after NGROUPS//2 per block
    cnt = [0, 0]
    for kind, b, sts in emit:
        prev = cnt[b]
        cnt[b] += len(sts) * GPS
        if splits[b] is None and cnt[b] >= NGROUPS - 512:
            splits[b] = cnt[b]
    return emit, group_map, splits


def _build_bass(fast):
    import concourse.bacc as bacc
    import concourse.mybir as mybir
    from concourse.tile import TileContext

    f32 = mybir.dt.float32
    f16 = mybir.dt.float16
    u32 = mybir.dt.uint32
    mm_dt = f16 if fast else f32

    emit, group_map, splits = _schedule()
    # per-block, per-supertile: ("tr",) or ("stage", tree_trigger_or_None)
    route = {}
    stage_of = {}
    for kind, b, sts in emit:
        if kind == "tr":
            route[(b, sts[0])] = ("tr", None)
        else:
            for st in sts:
                route[(b, st)] = ("stage", None)
            if len(sts) == RUN:
                # full runs emit their tree inline; partial runs flush at the
                # very end so emission order matches _schedule's group_map
                route[(b, sts[-1])] = ("stage", (sts, len(group_map)))
                stage_of[(b, sts[-1])] = sts

    nc = bacc.Bacc(
        "TRN2", target_bir_lowering=False, debug=False, num_devices=NCORES
    )
    qaugT = nc.declare_dram_parameter("qaugT", [DAUG, B], mm_dt, isOutput=False)
    memT = nc.declare_dram_parameter("memT", [DAUG, NSP], mm_dt, isOutput=False)
    out_gidx = nc.declare_dram_parameter(
        "out_gidx", [2, 2, 128, TOPG], u32, isOutput=True
    )

    with TileContext(nc) as tc:
        with (
            tc.tile_pool(name="const", bufs=1) as const_pool,
            tc.tile_pool(name="chunks", bufs=5) as chunk_pool,
            tc.tile_pool(name="psum", bufs=2, space="PSUM") as psum_pool,
            tc.tile_pool(name="rbuf", bufs=1) as r_pool,
            tc.tile_pool(name="ext", bufs=2) as ext_pool,
            tc.tile_pool(name="stg", bufs=3) as stg_pool,
            tc.tile_pool(name="tree", bufs=3) as tree_pool,
        ):
            # qaug weights replicated at partition bases 0 and 64 for the two
            # PE row-tiles (64x128 mode: T0 reads SBUF partitions 0-63, T8
            # reads 64-127).
            qaug_sb = const_pool.tile([128, B], mm_dt)
            nc.sync.dma_start(out=qaug_sb[0:DAUG, :], in_=qaugT[:, :])
            nc.sync.dma_start(out=qaug_sb[64 : 64 + DAUG, :], in_=qaugT[:, :])

            r0 = r_pool.tile([128, NGROUPS], f16, tag="r0")
            r1 = r_pool.tile([128, NGROUPS], f16, tag="r1")
            rs = [r0, r1]

            def emit_tree(buf, k, rout):
                # buf [128, k*CHUNK] f16 -> rout [128, k*GPS] group maxima
                ng = k * GPS
                v = buf[:, : k * CHUNK].rearrange("p (g w) -> p g w", w=G)
                t16 = tree_pool.tile([128, ng, 16], f16, tag="t16")
                t8 = tree_pool.tile([128, ng, 8], f16, tag="t8")
                t4 = tree_pool.tile([128, ng, 4], f16, tag="t4")
                t2 = tree_pool.tile([128, ng, 2], f16, tag="t2")
                nc.vector.tensor_max(t16, v[:, :, 0:16], v[:, :, 16:32])
                nc.vector.tensor_max(t8, t16[:, :, 0:8], t16[:, :, 8:16])
                nc.vector.tensor_max(t4, t8[:, :, 0:4], t8[:, :, 4:8])
                nc.vector.tensor_max(t2, t4[:, :, 0:2], t4[:, :, 2:4])
                nc.vector.tensor_max(rout, t2[:, :, 0], t2[:, :, 1])

            def emit_extract(b, h, lo, hi):
                width = hi - lo
                rsl = rs[b][:, lo:hi]
                v1 = ext_pool.tile([128, 8], f16)
                i1 = ext_pool.tile([128, 8], u32)
                rm = ext_pool.tile([128, width], f16, tag=f"rm")
                v2 = ext_pool.tile([128, 8], f16)
                i2 = ext_pool.tile([128, 8], u32)
                nc.vector.max(out=v1, in_=rsl)
                nc.vector.max_index(out=i1, in_max=v1, in_values=rsl)
                nc.vector.match_replace(
                    out=rm, in_to_replace=v1, in_values=rsl, imm_value=NEG_REP
                )
                nc.vector.max(out=v2, in_=rm[:])
                nc.vector.max_index(out=i2, in_max=v2, in_values=rm[:])
                nc.sync.dma_start(out=out_gidx[b, h, :, 0:8], in_=i1)
                nc.sync.dma_start(out=out_gidx[b, h, :, 8:16], in_=i2)

            rcol = [0, 0]
            extracted1 = [False, False]
            stage = [None, None]
            stage_base = [0, 0]
            for st in range(NSUPER):
                if st % 2 == 0:
                    chunk2 = chunk_pool.tile([128, 2 * CHUNK], mm_dt)
                    span = min(2 * CHUNK, NSP - st * CHUNK)
                    src = memT[:, st * CHUNK : st * CHUNK + span]
                    nc.sync.dma_start(out=chunk2[0:DAUG, :span], in_=src)
                    nc.sync.dma_start(out=chunk2[64 : 64 + DAUG, :span], in_=src)
                coff = (st % 2) * CHUNK
                psum0 = psum_pool.tile([128, CHUNK], f32, tag="psum0")
                psum1 = psum_pool.tile([128, CHUNK], f32, tag="psum1")
                psums = [psum0, psum1]
                # Interleave the two PE row-tiles so their matmuls overlap.
                for i in range(CHUNK // 512):
                    for b in range(2):
                        base = b * 64
                        nc.tensor.matmul(
                            psums[b][:, i * 512 : (i + 1) * 512],
                            lhsT=qaug_sb[base : base + DAUG, b * 128 : (b + 1) * 128],
                            rhs=chunk2[
                                base : base + DAUG,
                                coff + i * 512 : coff + (i + 1) * 512,
                            ],
                            start=True,
                            stop=True,
                            tile_position=(base, 0),
                        )
                for b in range(2):
                    kind, trig = route[(b, st)]
                    if kind == "tr":
                        nc.vector.tensor_reduce(
                            out=rs[b][:, rcol[b] : rcol[b] + GPS],
                            in_=psums[b][:].rearrange("p (g w) -> p g w", w=G),
                            axis=mybir.AxisListType.X,
                            op=mybir.AluOpType.max,
                        )
                        rcol[b] += GPS
                    else:
                        if stage[b] is None:
                            stg = stg_pool.tile(
                                [128, RUN * CHUNK], f16, tag=f"stage{b}"
                            )
                            stage[b] = stg
                            stage_base[b] = 0
                        nc.scalar.copy(
                            out=stage[b][
                                :,
                                stage_base[b] * CHUNK : (stage_base[b] + 1) * CHUNK,
                            ],
                            in_=psums[b][:],
                        )
                        stage_base[b] += 1
                        if trig is not None:
                            sts, _ = trig
                            k = len(sts)
                            emit_tree(
                                stage[b], k, rs[b][:, rcol[b] : rcol[b] + k * GPS]
                            )
                            rcol[b] += k * GPS
                            stage[b] = None
                    if not extracted1[b] and rcol[b] >= splits[b]:
                        emit_extract(b, 0, 0, splits[b])
                        extracted1[b] = True

            for b in range(2):
                # tail flush of a partial stage
                if stage[b] is not None:
                    k = stage_base[b]
                    emit_tree(stage[b], k, rs[b][:, rcol[b] : rcol[b] + k * GPS])
                    rcol[b] += k * GPS
                    stage[b] = None
                assert rcol[b] == NGROUPS, (b, rcol[b])
                if not extracted1[b]:
                    emit_extract(b, 0, 0, splits[b])
                emit_extract(b, 1, splits[b], NGROUPS)
    nc.compile()
    return nc


def _get_bass(fast):
    key = ("nc", fast)
    if key not in _CACHE:
        _CACHE[key] = _build_bass(fast)
    return _CACHE[key]


def _run_device(qaugT, mem_shards, fast=True, trace=False):
    from concourse.bass_utils import run_bass_kernel_spmd

    nc = _get_bass(fast)
    in_maps = [{"qaugT": qaugT, "memT": mt} for mt in mem_shards]
    res = run_bass_kernel_spmd(nc, in_maps, core_ids=list(range(NCORES)), trace=trace)
    return res


def _prep_inputs(obs, action, mem_sa, fp16=True):
    q = np.concatenate([obs, action], axis=1).astype(np.float32)  # [256, 34]
    qaugT = np.zeros([DAUG, B], np.float32)
    qaugT[:D] = (2.0 * q).T
    qaugT[D] = 1.0

    mem_shards = []
    for c in range(NCORES):
        ms = mem_sa[c * NS : (c + 1) * NS].astype(np.float32)  # [NS, 34]
        mt = np.zeros([DAUG, NSP], np.float32)
        mt[:D, :NS] = ms.T
        mt[D, :NS] = -np.sum(ms.astype(np.float64) ** 2, axis=1).astype(np.float32)
        mt[D, NS:] = NEG_PAD
        mem_shards.append(mt)
    if fp16:
        with np.errstate(over="ignore"):
            qaugT = qaugT.astype(np.float16)
            mem_shards = [mt.astype(np.float16) for mt in mem_shards]
    return q, qaugT, mem_shards


def _candidate_rows(gidx_all):
    """gidx_all: [NCORES, 2, 2, 128, TOPG] raw device output (r positions).

    Returns rows [256, NCORES*2*TOPG*G] of global row ids (invalid -> -1).
    """
    _, group_map, splits = _schedule()
    gm = np.asarray(group_map, dtype=np.int64)  # [2, NGROUPS]
    off = np.asarray(splits, dtype=np.int64)  # [2]

    # positions -> r columns (second half is split-relative) -> group ids
    g = gidx_all.astype(np.int64)  # [c, b, h, p, k]
    half = np.arange(2)[None, None, :, None, None]
    g = g + np.where(half == 1, off[None, :, None, None, None], 0)
    bidx = np.arange(2)[None, :, None, None, None]
    gid = gm[bidx, g]  # [NCORES, 2, 2, 128, TOPG]

    offs = np.arange(G, dtype=np.int64)
    cols = gid[..., None] * G + offs  # [c, b, h, p, k, G] local padded cols
    valid = cols < NS
    rows = cols + (np.arange(NCORES, dtype=np.int64) * NS)[
        :, None, None, None, None, None
    ]
    rows = np.where(valid, rows, -1)
    # query index = b*128 + p
    rows = rows.transpose(1, 3, 0, 2, 4, 5).reshape(B, -1)
    return rows


def _host_finish(q, mem_sa, mem_Q, gidx_all):
    rows = _candidate_rows(gidx_all)
    valid = rows >= 0
    rows_safe = np.where(valid, rows, 0)

    mem64 = mem_sa.astype(np.float64)
    q64 = q.astype(np.float64)
    Qv = mem_Q[:, 0].astype(np.float64)

    out = np.empty(B, np.float64)
    CQ = 32
    for s in range(0, B, CQ):
        e = min(s + CQ, B)
        r = rows_safe[s:e]
        mm = mem64[r]  # [cq, M, 34]
        diff = mm - q64[s:e, None, :]
        d2 = np.einsum("qmd,qmd->qm", diff, diff)
        d2 = np.where(valid[s:e], d2, np.inf)
        part = np.argpartition(d2, 10, axis=1)[:, :10]
        dsel = np.take_along_axis(d2, part, axis=1)
        rsel = np.take_along_axis(r, part, axis=1)
        w = np.exp(dsel - dsel.max(axis=1, keepdims=True))
        w /= w.sum(axis=1, keepdims=True)
        out[s:e] = (w * Qv[rsel]).sum(axis=1)
    return out.astype(np.float32)


def kernel(obs, action, mem_sa, mem_Q):
    obs = np.asarray(obs)
    action = np.asarray(action)
    mem_sa = np.asarray(mem_sa)
    mem_Q = np.asarray(mem_Q)
    q, qaugT, mem_shards = _prep_inputs(obs, action, mem_sa)
    res = _run_device(qaugT, mem_shards, fast=True)
    gidx_all = np.stack(
        [np.asarray(r["out_gidx"]).reshape(2, 2, 128, TOPG) for r in res.results]
    )  # [NCORES, 2, 2, 128, TOPG]
    return _host_finish(q, mem_sa, mem_Q, gidx_all)


# revision 41
# speedup vs baseline: 1.0074x; 1.0074x over previous
"""
Distributed brute-force kNN memory-critic kernel for Trainium2 (8 NeuronCores).

Math:
  reference computes, per query q (B=256), squared L2 distances to N=500000
  memory rows m (D=34), takes the 10 nearest, then softmax over the raw
  distances weighting mem_Q values.

  d2(q,m) = |q|^2 - 2 q.m + |m|^2.
  Both top-k selection and softmax(dists) are invariant to the per-query
  constant |q|^2, so the device works with the *negated shifted score*
      s'(q,m) = 2 q.m - |m|^2        (larger s' == closer neighbor)
  computed as one matmul with augmented operands qaug=[2q,1], maug=[m,-|m|^2].

Device (per core, 62500-row shard padded to 63488 cols):
  - fp16 operands, PE 64x128 row-tiling: query block 0 on tile T0
    (SBUF partitions 0-35), block 1 on T8 (64-99), matmuls interleaved so the
    two array tiles overlap.  Scores accumulate in fp32 PSUM.
  - drain/reduce: each [128, 1024] PSUM tile is reduced to per-32-column
    group maxima, either directly on the vector engine (segmented
    tensor_reduce) or via scalar-engine fp16 copy + vector fp16 max tree
    (2 elem/cycle), scheduled so ACT and DVE run concurrently.
  - per query, top-16 groups of each half of the group-max array are
    extracted with max8 / max_index / match_replace and DMA'd out.

Cover guarantee (exact): if x is among a core's true local top-10 scores in
some half, x's group max >= x >= (10th best of half) >= (10th largest group
max of the half), so x's group is inside that half's top-10 groups; we
return 16 per half.  The fp16 score error (~+-0.03) is far below the
measured 10th->17th group-max margin (>=0.67), verified on the fixed inputs.

Host epilogue: exactly rescore the candidate groups' rows in float64
(256 queries x 8 cores x 32 groups x 32 rows), exact global top-10,
softmax-weighted sum of mem_Q.  All 128M distance computations + the
10^8-element reduction happen on device; host does ~70M MACs of rescoring.
"""

import numpy as np

NCORES = 8
B = 256
D = 34
DAUG = 36
N = 500000
NS = N // NCORES          # 62500 rows per core
CHUNK = 512               # supertile columns (1 PSUM bank)
NSUPER = 124
NSP = NSUPER * CHUNK      # 63488 padded shard columns
G = 32                    # group width for the segmented max
NGROUPS = NSP // G        # 1984 groups per 128-query block
GPS = CHUNK // G          # groups per supertile (32)
RUN = 8                   # supertiles batched per fp16 tree pass
TOPG = 16                 # groups returned per query per half
NEG_PAD = -1.0e30         # score assigned to padding columns
NEG_REP = -3.0e38         # match_replace fill

_CACHE = {}


def _schedule():
    """Drain routing + r-column emission order; must match _build_bass.

    Returns (events, group_map, splits):
      events: list of ("tr", b, st) | ("tree", b, [sts])
      group_map[b]: list of NGROUPS global group ids in r-column order
      splits[b]: r column where the extraction is split in two halves
    """
    events = []
    for st in range(NSUPER):
        events.append(("tr", 0, st) if st % 6 != 0 else ("stage", 0, st))
        events.append(("stage", 1, st))

    group_map = [[], []]
    splits = [None, None]
    emit = []
    pend = [[], []]
    for ev in events:
        kind, b, st = ev
        if kind == "tr":
            emit.append(("tr", b, [st]))
        else:
            pend[b].append(st)
            if len(pend[b]) == RUN:
                emit.append(("tree", b, list(pend[b])))
                pend[b] = []
    for b in range(2):
        if pend[b]:
            emit.append(("tree", b, list(pend[b])))
    for kind, b, sts in emit:
        for st in sts:
            group_map[b].extend(range(st * GPS, (st + 1) * GPS))
    # split points: first emission boundary at/after NGROUPS//2 per block
    cnt = [0, 0]
    for kind, b, sts in emit:
        prev = cnt[b]
        cnt[b] += len(sts) * GPS
        if splits[b] is None and cnt[b] >= NGROUPS - 512:
            splits[b] = cnt[b]
    return emit, group_map, splits


def _build_bass(fast):
    import concourse.bacc as bacc
    import concourse.mybir as mybir
    from concourse.tile import TileContext

    f32 = mybir.dt.float32
    f16 = mybir.dt.float16
    u32 = mybir.dt.uint32
    mm_dt = f16 if fast else f32

    emit, group_map, splits = _schedule()
    # per-block, per-supertile: ("tr",) or ("stage", tree_trigger_or_None)
    route = {}
    stage_of = {}
    for kind, b, sts in emit:
        if kind == "tr":
            route[(b, sts[0])] = ("tr", None)
        else:
            for st in sts:
                route[(b, st)] = ("stage", None)
            if len(sts) == RUN:
                # full runs emit their tree inline; partial runs flush at the
                # very end so emission order matches _schedule's group_map
                route[(b, sts[-1])] = ("stage", (sts, len(group_map)))
                stage_of[(b, sts[-1])] = sts

    nc = bacc.Bacc(
        "TRN2", target_bir_lowering=False, debug=False, num_devices=NCORES
    )
    qaugT = nc.declare_dram_parameter("qaugT", [DAUG, B], mm_dt, isOutput=False)
    memT = nc.declare_dram_parameter("memT", [DAUG, NSP], mm_dt, isOutput=False)
    out_gidx = nc.declare_dram_parameter(
        "out_gidx", [2, 2, 128, TOPG], u32, isOutput=True
    )

    with TileContext(nc) as tc:
        with (
            tc.tile_pool(name="const", bufs=1) as const_pool,
            tc.tile_pool(name="chunks", bufs=5) as chunk_pool,
            tc.tile_pool(name="psum", bufs=4, space="PSUM") as psum_pool,
            tc.tile_pool(name="rbuf", bufs=1) as r_pool,
            tc.tile_pool(name="ext", bufs=2) as ext_pool,
            tc.tile_pool(name="stg", bufs=3) as stg_pool,
            tc.tile_pool(name="tree", bufs=3) as tree_pool,
        ):
            # qaug weights replicated at partition bases 0 and 64 for the two
            # PE row-tiles (64x128 mode: T0 reads SBUF partitions 0-63, T8
            # reads 64-127).
            qaug_sb = const_pool.tile([128, B], mm_dt)
            nc.sync.dma_start(out=qaug_sb[0:DAUG, :], in_=qaugT[:, :])
            nc.sync.dma_start(out=qaug_sb[64 : 64 + DAUG, :], in_=qaugT[:, :])

            r0 = r_pool.tile([128, NGROUPS], f16, tag="r0")
            r1 = r_pool.tile([128, NGROUPS], f16, tag="r1")
            rs = [r0, r1]

            def emit_tree(buf, k, rout):
                # buf [128, k*CHUNK] f16 -> rout [128, k*GPS] group maxima
                ng = k * GPS
                v = buf[:, : k * CHUNK].rearrange("p (g w) -> p g w", w=G)
                t16 = tree_pool.tile([128, ng, 16], f16, tag="t16")
                t8 = tree_pool.tile([128, ng, 8], f16, tag="t8")
                t4 = tree_pool.tile([128, ng, 4], f16, tag="t4")
                t2 = tree_pool.tile([128, ng, 2], f16, tag="t2")
                nc.vector.tensor_max(t16, v[:, :, 0:16], v[:, :, 16:32])
                nc.vector.tensor_max(t8, t16[:, :, 0:8], t16[:, :, 8:16])
                nc.vector.tensor_max(t4, t8[:, :, 0:4], t8[:, :, 4:8])
                nc.vector.tensor_max(t2, t4[:, :, 0:2], t4[:, :, 2:4])
                nc.vector.tensor_max(rout, t2[:, :, 0], t2[:, :, 1])

            def emit_extract(b, h, lo, hi):
                width = hi - lo
                rsl = rs[b][:, lo:hi]
                v1 = ext_pool.tile([128, 8], f16)
                i1 = ext_pool.tile([128, 8], u32)
                rm = ext_pool.tile([128, width], f16, tag=f"rm")
                v2 = ext_pool.tile([128, 8], f16)
                i2 = ext_pool.tile([128, 8], u32)
                nc.vector.max(out=v1, in_=rsl)
                nc.vector.max_index(out=i1, in_max=v1, in_values=rsl)
                nc.vector.match_replace(
                    out=rm, in_to_replace=v1, in_values=rsl, imm_value=NEG_REP
                )
                nc.vector.max(out=v2, in_=rm[:])
                nc.vector.max_index(out=i2, in_max=v2, in_values=rm[:])
                nc.sync.dma_start(out=out_gidx[b, h, :, 0:8], in_=i1)
                nc.sync.dma_start(out=out_gidx[b, h, :, 8:16], in_=i2)

            rcol = [0, 0]
            extracted1 = [False, False]
            stage = [None, None]
            stage_base = [0, 0]
            for st in range(NSUPER):
                if st % 4 == 0:
                    chunk2 = chunk_pool.tile([128, 4 * CHUNK], mm_dt)
                    span = min(4 * CHUNK, NSP - st * CHUNK)
                    src = memT[:, st * CHUNK : st * CHUNK + span]
                    nc.sync.dma_start(out=chunk2[0:DAUG, :span], in_=src)
                    nc.sync.dma_start(out=chunk2[64 : 64 + DAUG, :span], in_=src)
                coff = (st % 4) * CHUNK
                psum0 = psum_pool.tile([128, CHUNK], f32, tag="psum0")
                psum1 = psum_pool.tile([128, CHUNK], f32, tag="psum1")
                psums = [psum0, psum1]
                # Interleave the two PE row-tiles so their matmuls overlap.
                for i in range(CHUNK // 512):
                    for b in range(2):
                        base = b * 64
                        nc.tensor.matmul(
                            psums[b][:, i * 512 : (i + 1) * 512],
                            lhsT=qaug_sb[base : base + DAUG, b * 128 : (b + 1) * 128],
                            rhs=chunk2[
                                base : base + DAUG,
                                coff + i * 512 : coff + (i + 1) * 512,
                            ],
                            start=True,
                            stop=True,
                            tile_position=(base, 0),
                        )
                for b in range(2):
                    kind, trig = route[(b, st)]
                    if kind == "tr":
                        nc.vector.tensor_reduce(
                            out=rs[b][:, rcol[b] : rcol[b] + GPS],
                            in_=psums[b][:].rearrange("p (g w) -> p g w", w=G),
                            axis=mybir.AxisListType.X,
                            op=mybir.AluOpType.max,
                        )
                        rcol[b] += GPS
                    else:
                        if stage[b] is None:
                            stg = stg_pool.tile(
                                [128, RUN * CHUNK], f16, tag=f"stage{b}"
                            )
                            stage[b] = stg
                            stage_base[b] = 0
                        nc.scalar.copy(
                            out=stage[b][
                                :,
                                stage_base[b] * CHUNK : (stage_base[b] + 1) * CHUNK,
                            ],
                            in_=psums[b][:],
                        )
                        stage_base[b] += 1
                        if trig is not None:
                            sts, _ = trig
                            k = len(sts)
                            emit_tree(
                                stage[b], k, rs[b][:, rcol[b] : rcol[b] + k * GPS]
                            )
                            rcol[b] += k * GPS
                            stage[b] = None
                    if not extracted1[b] and rcol[b] >= splits[b]:
                        emit_extract(b, 0, 0, splits[b])
                        extracted1[b] = True

            for b in range(2):
                # tail flush of a partial stage
                if stage[b] is not None:
                    k = stage_base[b]
                    emit_tree(stage[b], k, rs[b][:, rcol[b] : rcol[b] + k * GPS])
                    rcol[b] += k * GPS
                    stage[b] = None
                assert rcol[b] == NGROUPS, (b, rcol[b])
                if not extracted1[b]:
                    emit_extract(b, 0, 0, splits[b])
                emit_extract(b, 1, splits[b], NGROUPS)
    nc.compile()
    return nc


def _get_bass(fast):
    key = ("nc", fast)
    if key not in _CACHE:
        _CACHE[key] = _build_bass(fast)
    return _CACHE[key]


def _run_device(qaugT, mem_shards, fast=True, trace=False):
    from concourse.bass_utils import run_bass_kernel_spmd

    nc = _get_bass(fast)
    in_maps = [{"qaugT": qaugT, "memT": mt} for mt in mem_shards]
    res = run_bass_kernel_spmd(nc, in_maps, core_ids=list(range(NCORES)), trace=trace)
    return res


def _prep_inputs(obs, action, mem_sa, fp16=True):
    q = np.concatenate([obs, action], axis=1).astype(np.float32)  # [256, 34]
    qaugT = np.zeros([DAUG, B], np.float32)
    qaugT[:D] = (2.0 * q).T
    qaugT[D] = 1.0

    mem_shards = []
    for c in range(NCORES):
        ms = mem_sa[c * NS : (c + 1) * NS].astype(np.float32)  # [NS, 34]
        mt = np.zeros([DAUG, NSP], np.float32)
        mt[:D, :NS] = ms.T
        mt[D, :NS] = -np.sum(ms.astype(np.float64) ** 2, axis=1).astype(np.float32)
        mt[D, NS:] = NEG_PAD
        mem_shards.append(mt)
    if fp16:
        with np.errstate(over="ignore"):
            qaugT = qaugT.astype(np.float16)
            mem_shards = [mt.astype(np.float16) for mt in mem_shards]
    return q, qaugT, mem_shards


def _candidate_rows(gidx_all):
    """gidx_all: [NCORES, 2, 2, 128, TOPG] raw device output (r positions).

    Returns rows [256, NCORES*2*TOPG*G] of global row ids (invalid -> -1).
    """
    _, group_map, splits = _schedule()
    gm = np.asarray(group_map, dtype=np.int64)  # [2, NGROUPS]
    off = np.asarray(splits, dtype=np.int64)  # [2]

    # positions -> r columns (second half is split-relative) -> group ids
    g = gidx_all.astype(np.int64)  # [c, b, h, p, k]
    half = np.arange(2)[None, None, :, None, None]
    g = g + np.where(half == 1, off[None, :, None, None, None], 0)
    bidx = np.arange(2)[None, :, None, None, None]
    gid = gm[bidx, g]  # [NCORES, 2, 2, 128, TOPG]

    offs = np.arange(G, dtype=np.int64)
    cols = gid[..., None] * G + offs  # [c, b, h, p, k, G] local padded cols
    valid = cols < NS
    rows = cols + (np.arange(NCORES, dtype=np.int64) * NS)[
        :, None, None, None, None, None
    ]
    rows = np.where(valid, rows, -1)
    # query index = b*128 + p
    rows = rows.transpose(1, 3, 0, 2, 4, 5).reshape(B, -1)
    return rows


def _host_finish(q, mem_sa, mem_Q, gidx_all):
    rows = _candidate_rows(gidx_all)
    valid = rows >= 0
    rows_safe = np.where(valid, rows, 0)

    mem64 = mem_sa.astype(np.float64)
    q64 = q.astype(np.float64)
    Qv = mem_Q[:, 0].astype(np.float64)

    out = np.empty(B, np.float64)
    CQ = 32
    for s in range(0, B, CQ):
        e = min(s + CQ, B)
        r = rows_safe[s:e]
        mm = mem64[r]  # [cq, M, 34]
        diff = mm - q64[s:e, None, :]
        d2 = np.einsum("qmd,qmd->qm", diff, diff)
        d2 = np.where(valid[s:e], d2, np.inf)
        part = np.argpartition(d2, 10, axis=1)[:, :10]
        dsel = np.take_along_axis(d2, part, axis=1)
        rsel = np.take_along_axis(r, part, axis=1)
        w = np.exp(dsel - dsel.max(axis=1, keepdims=True))
        w /= w.sum(axis=1, keepdims=True)
        out[s:e] = (w * Qv[rsel]).sum(axis=1)
    return out.astype(np.float32)


def kernel(obs, action, mem_sa, mem_Q):
    obs = np.asarray(obs)
    action = np.asarray(action)
    mem_sa = np.asarray(mem_sa)
    mem_Q = np.asarray(mem_Q)
    q, qaugT, mem_shards = _prep_inputs(obs, action, mem_sa)
    res = _run_device(qaugT, mem_shards, fast=True)
    gidx_all = np.stack(
        [np.asarray(r["out_gidx"]).reshape(2, 2, 128, TOPG) for r in res.results]
    )  # [NCORES, 2, 2, 128, TOPG]
    return _host_finish(q, mem_sa, mem_Q, gidx_all)
